# revision 21
# baseline (speedup 1.0000x reference)
"""AttnVLAD layer on 8 Trainium2 NeuronCores.

Data-parallel over batch: b=32 samples -> 4 per core. Params
(centers/alpha/cluster_weights) replicated. Per sample:
  scoreT[n,K] = (alpha * centers/||centers||)^T-weighted matmul (fp32)
  prob = softmax over K (fp16 output)
  descT[K,d] = prob^T @ x^T (fp16 matmuls, fp32 PSUM accum)
  epilogue: denom-normalize, subtract centers, intra-L2, cluster
  weights, global L2 -> out[d*K]
"""
import numpy as np

B, D, N, K = 32, 512, 4096, 64
NCORES = 8
SPC = B // NCORES          # samples per core
DCH = D // 128             # 4 d-chunks
NCH = N // 128             # 32 n-chunks

_COMPILED = {}


def _build():
    import concourse.bass as bass
    import concourse.bacc as bacc
    import concourse.tile as tile
    import concourse.mybir as mybir

    f32 = mybir.dt.float32
    f16 = mybir.dt.float16
    AF = mybir.ActivationFunctionType
    OP = mybir.AluOpType
    AX = mybir.AxisListType

    nc = bacc.Bacc("TRN2", target_bir_lowering=False, debug=False)
    xc_dram = nc.dram_tensor("xc", [SPC, D, N], f32, kind="ExternalInput")
    c_dram = nc.dram_tensor("centers", [D, K], f32, kind="ExternalInput")
    alpha_dram = nc.dram_tensor("alpha", [1, 1], f32, kind="ExternalInput")
    cw_dram = nc.dram_tensor("cw", [K, 1], f32, kind="ExternalInput")
    id_dram = nc.dram_tensor("ident", [128, 128], f32, kind="ExternalInput")
    out_dram = nc.dram_tensor("out", [SPC, D * K], f32, kind="ExternalOutput")

    with tile.TileContext(nc) as tc:
        with (
            tc.tile_pool(name="const", bufs=1) as const,
            tc.tile_pool(name="xpool", bufs=2) as xpool,
            tc.tile_pool(name="xhp", bufs=5) as xhp,
            tc.tile_pool(name="xlp", bufs=5) as xlp,
            tc.tile_pool(name="xTp", bufs=1) as xTp,
            tc.tile_pool(name="probp", bufs=2) as probp,
            tc.tile_pool(name="smp", bufs=8) as smp,
            tc.tile_pool(name="epp", bufs=1) as epp,
            tc.tile_pool(name="ps_sc", bufs=3, space="PSUM") as ps_sc,
            tc.tile_pool(name="ps_d", bufs=2, space="PSUM") as ps_d,
            tc.tile_pool(name="ps_n", bufs=1, space="PSUM") as ps_n,
            tc.tile_pool(name="ps_m", bufs=2, space="PSUM") as ps_m,
        ):
            # ---------- one-time prep ----------
            ident = const.tile([128, 128], f32, tag="ident")
            nc.sync.dma_start(ident[:], id_dram[:])
            c_sb = const.tile([128, DCH, K], f32, tag="c_sb")
            nc.sync.dma_start(
                c_sb[:], c_dram[:].rearrange("(c p) k -> p c k", p=128))
            alpha_sb = const.tile([1, 1], f32, tag="alpha_sb")
            nc.sync.dma_start(alpha_sb[:], alpha_dram[:])
            cw_sb = const.tile([K, 1], f32, tag="cw_sb")
            nc.sync.dma_start(cw_sb[:], cw_dram[:])
            ones16 = const.tile([128, 1], f16, tag="ones16")
            nc.gpsimd.memset(ones16[:], 1.0)
            onesc = const.tile([128, 1], f32, tag="onesc")
            nc.gpsimd.memset(onesc[:], 1.0)
            onesr = const.tile([1, 128], f32, tag="onesr")
            nc.gpsimd.memset(onesr[:], 1.0)

            # q_s = centers * (alpha / max(||c||_d, 1e-12)), fp32 [128, DCH, K]
            sq = const.tile([128, DCH, K], f32, tag="sq")
            nc.vector.tensor_mul(sq[:], c_sb[:], c_sb[:])
            ssum = ps_m.tile([1, K], f32, tag="m")
            for dc in range(DCH):
                nc.tensor.matmul(ssum[:], onesc[:], sq[:, dc, :],
                                 start=(dc == 0), stop=(dc == DCH - 1))
            cnorm = const.tile([1, K], f32, tag="cnorm")
            nc.scalar.activation(cnorm[:], ssum[:], AF.Sqrt)
            nc.vector.tensor_scalar_max(cnorm[:], cnorm[:], 1e-12)
            cscale = const.tile([1, K], f32, tag="cscale")
            nc.vector.reciprocal(cscale[:], cnorm[:])
            nc.vector.tensor_scalar_mul(cscale[:], cscale[:], alpha_sb[:])
            scale_rep = ps_m.tile([128, K], f32, tag="m")
            nc.tensor.matmul(scale_rep[:], onesr[:], cscale[:],
                             start=True, stop=True)
            q_s = const.tile([128, DCH, K], f32, tag="q_s")
            for dc in range(DCH):
                nc.vector.tensor_mul(q_s[:, dc, :], c_sb[:, dc, :],
                                     scale_rep[:])
            # fp16 split of q_s for the 3-term mm1
            qh = const.tile([128, DCH, K], f16, tag="qh")
            nc.vector.tensor_copy(qh[:], q_s[:])
            ql = const.tile([128, DCH, K], f16, tag="ql")
            nc.vector.tensor_sub(ql[:], q_s[:], qh[:])

            # centersT [K, D] for the epilogue subtract
            cT = const.tile([K, D], f32, tag="cT")
            for dc in range(DCH):
                tp = ps_m.tile([K, 128], f32, tag="m")
                nc.tensor.transpose(tp[:], c_sb[:, dc, :], ident[:])
                nc.scalar.copy(cT[:, dc * 128:(dc + 1) * 128], tp[:])

            # ---------- per-sample pipeline ----------
            # per chunk: load fp32, cast to xh=f16(x), compute xl=f16(x-xh)
            # (3-term fp16 split: q^T x ~= qh^T xh + ql^T xh + qh^T xl)
            def load_split(s, dc):
                x32 = xpool.tile([128, N], f32, tag="x32",
                                 name=f"x32_{s}_{dc}")
                nc.sync.dma_start(x32[:], xc_dram[s, dc * 128:(dc + 1) * 128, :])
                xh = xhp.tile([128, N], f16, tag="xh", name=f"xh_{s}_{dc}")
                if dc == 0:
                    nc.vector.tensor_copy(xh[:], x32[:])
                else:
                    nc.scalar.copy(xh[:], x32[:])
                xl = xlp.tile([128, N], f16, tag="xl", name=f"xl_{s}_{dc}")
                if dc == DCH - 1:
                    nc.gpsimd.tensor_sub(xl[:], x32[:], xh[:])
                else:
                    nc.vector.tensor_sub(xl[:], x32[:], xh[:])
                return xh, xl

            # prefetch sample 0
            xcur = [load_split(0, dc) for dc in range(DCH)]

            for s in range(SPC):
                # transpose xh -> xT16 [p, dc, j, 128] for mm2
                xT16 = xTp.tile([128, DCH, NCH, 128], f16, tag="xT16")
                for dc in range(DCH):
                    nc.sync.dma_start_transpose(xT16[:, dc, :, :],
                                                xcur[dc][0][:])

                descT = ps_d.tile([K, D], f32, tag="descT")
                denom = ps_n.tile([K, 1], f32, tag="denom")
                probs = probp.tile([128, NCH, K], f16, tag="prob")
                xnext = [None] * DCH
                LAG = 4
                BPB = 8  # score chunks per PSUM bank (one accum group/bank)

                NB = NCH // BPB  # score banks per sample

                def mm1_bank(b):
                    bank = ps_sc.tile([128, BPB, K], f32, tag="scoreT",
                                      name=f"scb_{s}_{b}")
                    first = [True]

                    def mm(c, lhsT, rhs, last=False):
                        nc.tensor.matmul(
                            bank[:, c, :], lhsT, rhs,
                            start=first[0], stop=last,
                            skip_group_check=(not first[0]))
                        first[0] = False

                    for c in range(BPB):
                        j = b * BPB + c
                        sl = slice(j * 128, (j + 1) * 128)
                        for dc in range(DCH):
                            xh, xl = xcur[dc]
                            mm(c, xh[:, sl], qh[:, dc, :])
                            mm(c, xh[:, sl], ql[:, dc, :])
                            mm(c, xl[:, sl], qh[:, dc, :],
                               last=(c == BPB - 1 and dc == DCH - 1))
                    return bank

                def softmax_bank(b, bank):
                    # segmented softmax over K for 8 chunks at once
                    negmax = smp.tile([128, BPB], f32, tag="negmax")
                    nc.vector.reduce_max(negmax[:].unsqueeze(2),
                                         bank[:], axis=AX.X, negate=True)
                    e16 = smp.tile([128, BPB, K], f16, tag="e16")
                    for c in range(BPB):
                        nc.scalar.activation(e16[:, c, :], bank[:, c, :],
                                             AF.Exp, bias=negmax[:, c:c + 1])
                    rs = smp.tile([128, BPB], f32, tag="rs")
                    nc.vector.reduce_sum(rs[:].unsqueeze(2), e16[:], axis=AX.X)
                    rr = smp.tile([128, BPB], f32, tag="rr")
                    nc.vector.reciprocal(rr[:], rs[:])
                    nc.vector.tensor_mul(
                        probs[:, b * BPB:(b + 1) * BPB, :], e16[:],
                        rr[:].unsqueeze(2).broadcast_to([128, BPB, K]))

                def mm2_bank(b):
                    for c in range(BPB):
                        j = b * BPB + c
                        nc.tensor.matmul(descT[:], probs[:, j, :],
                                         xT16[:, :, j, :],
                                         start=(j == 0), stop=(j == NCH - 1))
                    for c in range(BPB):
                        j = b * BPB + c
                        nc.tensor.matmul(denom[:], probs[:, j, :], ones16[:],
                                         start=(j == 0), stop=(j == NCH - 1))

                for b in range(NB):
                    bank = mm1_bank(b)
                    softmax_bank(b, bank)
                    if b >= 2:
                        mm2_bank(b - 2)
                    # chunk-wise prefetch of the next sample (2 per bank)
                    if s + 1 < SPC and b < 2:
                        xnext[2 * b] = load_split(s + 1, 2 * b)
                        xnext[2 * b + 1] = load_split(s + 1, 2 * b + 1)
                mm2_bank(NB - 2)
                mm2_bank(NB - 1)

                if s + 1 < SPC:
                    xcur = xnext

                # ---------- epilogue (descT [K, D] layout) ----------
                rdenom = epp.tile([K, 1], f32, tag="rdenom")
                nc.vector.tensor_scalar_max(rdenom[:], denom[:], 1e-6)
                nc.vector.reciprocal(rdenom[:], rdenom[:])
                desc_c = epp.tile([K, D], f32, tag="desc_c")
                nc.vector.scalar_tensor_tensor(
                    desc_c[:], in0=descT[:], scalar=rdenom[:], in1=cT[:],
                    op0=OP.mult, op1=OP.subtract)
                sqe = epp.tile([K, D], f32, tag="sqe")
                nc.vector.tensor_mul(sqe[:], desc_c[:], desc_c[:])
                ss = epp.tile([K, 1], f32, tag="ss")
                nc.vector.reduce_sum(ss[:], sqe[:], axis=AX.X)
                intra = epp.tile([K, 1], f32, tag="intra")
                nc.scalar.activation(intra[:], ss[:], AF.Sqrt)
                nc.vector.tensor_scalar_max(intra[:], intra[:], 1e-12)
                rintra = epp.tile([K, 1], f32, tag="rintra")
                nc.vector.reciprocal(rintra[:], intra[:])
                cwr = epp.tile([K, 1], f32, tag="cwr")
                nc.vector.tensor_mul(cwr[:], cw_sb[:], rintra[:])
                # t = ss * cwr^2 ; total = sum_k t
                t1 = epp.tile([K, 1], f32, tag="t1")
                nc.vector.tensor_mul(t1[:], ss[:], cwr[:])
                nc.vector.tensor_mul(t1[:], t1[:], cwr[:])
                tot = ps_m.tile([1, 1], f32, tag="m")
                nc.tensor.matmul(tot[:], t1[:], onesc[:K, :],
                                 start=True, stop=True)
                fin = epp.tile([1, 1], f32, tag="fin")
                nc.scalar.activation(fin[:], tot[:], AF.Sqrt)
                nc.vector.tensor_scalar_max(fin[:], fin[:], 1e-12)
                nc.vector.reciprocal(fin[:], fin[:])
                finrep = ps_m.tile([K, 1], f32, tag="m")
                nc.tensor.matmul(finrep[:], onesr[:, :K], fin[:],
                                 start=True, stop=True)
                sfin = epp.tile([K, 1], f32, tag="sfin")
                nc.vector.tensor_mul(sfin[:], cwr[:], finrep[:])
                outT = epp.tile([K, D], f32, tag="outT")
                nc.vector.tensor_mul(outT[:], desc_c[:],
                                     sfin[:].broadcast_to([K, D]))
                # transpose [K, D] -> [D, K] in 128-col blocks, DMA out
                for dc in range(DCH):
                    tp = ps_m.tile([128, K], f32, tag="m")
                    nc.tensor.transpose(
                        tp[:], outT[:, dc * 128:(dc + 1) * 128],
                        ident[:K, :K])
                    ot = epp.tile([128, K], f32, tag="ot")
                    nc.scalar.copy(ot[:], tp[:])
                    nc.sync.dma_start(
                        out_dram[s, dc * 128 * K:(dc + 1) * 128 * K]
                        .rearrange("(p k) -> p k", k=K),
                        ot[:])

    nc.compile()
    return nc


def kernel(x, centers, alpha, cluster_weights):
    import concourse.bass_utils as bass_utils

    if "nc" not in _COMPILED:
        _COMPILED["nc"] = _build()
    nc = _COMPILED["nc"]

    x = np.ascontiguousarray(np.asarray(x, dtype=np.float32))
    c = np.asarray(centers, dtype=np.float32).reshape(D, K)
    a = np.asarray(alpha, dtype=np.float32).reshape(1, 1)
    cw = np.asarray(cluster_weights, dtype=np.float32).reshape(K, 1)
    ident = np.eye(128, dtype=np.float32)

    in_maps = []
    for core in range(NCORES):
        in_maps.append({
            "xc": x[core * SPC:(core + 1) * SPC],
            "centers": c,
            "alpha": a,
            "cw": cw,
            "ident": ident,
        })
    res = bass_utils.run_bass_kernel_spmd(nc, in_maps,
                                          core_ids=list(range(NCORES)))
    out = np.concatenate([res.results[i]["out"] for i in range(NCORES)],
                         axis=0)
    return out.astype(np.float32)
